# revision 9
# baseline (speedup 1.0000x reference)
"""CoAttention kernel for 8x TRN2 NeuronCores.

Computation (per batch b):
    q = x[b] @ Wq.T + bq            [Sq, H]
    k = y[b] @ Wk.T + bk            [Skv, H]
    v = y[b] @ Wv.T + bv            [Skv, H]
    out[b] = softmax(q @ k.T / sqrt(H)) @ v

Sharding: data-parallel over batch; each of the 8 cores handles B/8 = 2
batches. Weights are replicated. Host staging transposes activations to
[D, S] (contraction dim on partitions) and casts matmul operands to fp16
(PE runs fp16 at 4x the fp32 rate; fp32 accumulation in PSUM keeps the
absmax-relative error ~4e-4, verified against a float64 reference).

Device-side layout choices:
  - Q^T [H, Sq] and K^T [H, Skv] (H on partitions) so the score matmul
    contracts over H, and the per-partition bias add is free on DVE.
  - Scores are built TRANSPOSED: S^T[t, s] = (K^T tile).T @ Q^T, so that
    P^T = exp(S^T) is directly usable as the stationary operand of the
    P @ V matmul (contraction over t on partitions).
  - Softmax denominator comes for free as a ones-column appended to V:
    out_psum[:, H] = sum_t P^T[t, s]. No max-subtraction is needed:
    logits are O(1) here, exp cannot overflow, and softmax is shift-
    invariant so the result matches the reference exactly.
  - bv is folded past the softmax: rows of softmax sum to 1, so
    out = (P @ v_raw) / denom + bv.

Single-shot (harness-metric) optimizations over the plain pipeline:
  - Batch 0 runs its Q and K projections k-OUTER over fine-grained input
    DMA chunks: 4 psum accumulators per wave consume each arriving
    [128, 1, S] k-slice (853ns of PE work per 790ns of wire), so the PE
    never stalls waiting for x/y to finish landing. Later batches are
    prefetched under compute and keep the cheaper k-inner order.
  - The last s-block of the last batch runs P@V j-MAJOR (each 128-row
    output strip finishes its full t-accumulation, epilogue, and output
    DMA before the next starts), cutting the end-of-kernel drain from
    ~4us to ~2us.
  - Matmuls issued in the first ~3.4us run at reduced clock (pstate
    ramp), so that window is burned on dummy matmuls while the first
    input chunks are still in flight.
"""

import os
import sys
from contextlib import ExitStack

import numpy as np

sys.path.insert(0, "/opt/trn_rl_repo")

N_CORES = 8
B, SQ, SKV, D, H = 16, 1024, 1024, 768, 256
BL = B // N_CORES  # batches per core
KD = D // 128      # 6 contraction tiles for the projections
JH = H // 128      # 2 partition tiles of hidden
TS = SKV // 128    # 8 kv tiles
SB = SQ // 512     # 2 query blocks of 512

_cached = {}


def _env(name, default):
    return int(os.environ.get(name, str(default)))


def _build_nc(reps=1):
    import concourse.bass as bass
    import concourse.tile as tile
    from concourse import bacc, mybir

    f16 = mybir.dt.float16
    f32 = mybir.dt.float32
    Exp = mybir.ActivationFunctionType.Exp
    Copy = mybir.ActivationFunctionType.Copy
    mult = mybir.AluOpType.mult
    add = mybir.AluOpType.add

    nc = bacc.Bacc("TRN2", target_bir_lowering=False, debug=False)

    xT = nc.dram_tensor("xT", [BL, D, SQ], f16, kind="ExternalInput")
    yT = nc.dram_tensor("yT", [BL, D, SKV], f16, kind="ExternalInput")
    wqT = nc.dram_tensor("wqT", [D, H], f16, kind="ExternalInput")
    wkT = nc.dram_tensor("wkT", [D, H], f16, kind="ExternalInput")
    wvT = nc.dram_tensor("wvT", [D, H], f16, kind="ExternalInput")
    # biases packed host-side into one tensor -> one DMA (HWDGE descriptor
    # generation is ~0.6us per dma_start regardless of size):
    # cols [0:JH]=bq tiles, [JH:2*JH]=bk tiles, [2*JH:2*JH+H]=bv broadcast.
    biasd = nc.dram_tensor("biases", [128, 2 * JH + H], f32, kind="ExternalInput")
    outd = nc.dram_tensor("out", [BL, SQ, H], f32, kind="ExternalOutput")

    with tile.TileContext(nc) as tc, ExitStack() as ctx:
        wpool = ctx.enter_context(tc.tile_pool(name="w", bufs=1))
        cpool = ctx.enter_context(tc.tile_pool(name="c", bufs=1))
        xpool = ctx.enter_context(tc.tile_pool(name="acts", bufs=2))
        qkv = ctx.enter_context(tc.tile_pool(name="qkv", bufs=2))
        ptp = ctx.enter_context(
            tc.tile_pool(name="ptp", bufs=_env("KERNEL_PTP_BUFS", 9 if reps == 1 else 6))
        )
        outp = ctx.enter_context(tc.tile_pool(name="outp", bufs=4))
        smallp = ctx.enter_context(tc.tile_pool(name="small", bufs=4))
        # reps==1 (the single-shot build the harness profiles): one shared
        # [128, 512] psum pool covers the 4-acc k-outer projection waves of
        # batch 0, later batches' rotating accumulators, and the S^T tiles.
        # reps>1 (the slope-benchmark build): keep the proven baseline pool
        # split — the merged ring interacts badly with For_i scheduling.
        if reps == 1:
            psAS = ctx.enter_context(
                tc.tile_pool(name="psAS", bufs=4, space=bass.MemorySpace.PSUM)
            )
            psProj, proj_tag = psAS, "st"
        else:
            psProj = ctx.enter_context(
                tc.tile_pool(name="psA", bufs=2, space=bass.MemorySpace.PSUM)
            )
            proj_tag = "proj"
            psAS = ctx.enter_context(
                tc.tile_pool(name="psS", bufs=2, space=bass.MemorySpace.PSUM)
            )
        psO = ctx.enter_context(
            tc.tile_pool(
                name="psO",
                bufs=_env("KERNEL_PSO_BUFS", 4),
                space=bass.MemorySpace.PSUM,
            )
        )

        # The first real matmul can't start until wq + the first x chunk
        # land. Matmuls issued in the first ~3.4us of PE activity run at
        # reduced clock (pstate ramp), so burn that window on dummy
        # matmuls over zeroed scratch.
        nwarm = _env("KERNEL_WARMUP_MMS", 8)
        wcols = _env("KERNEL_WARMUP_COLS", 512)
        if nwarm:
            warm_sb = cpool.tile([128, wcols], f16, tag="warm")
            nc.vector.memset(warm_sb[:], 0.0)
            warm_ps = psAS.tile([128, 512], f32, tag="st", name="warm_ps")
            for _ in range(nwarm):
                nc.tensor.matmul(
                    warm_ps[:, 0:wcols], warm_sb[:, 0:128], warm_sb[:],
                    start=True, stop=True,
                )

        # Replicated constants. Every dma_start pays ~0.6us of serialized
        # HWDGE descriptor generation, so batch 0 is issued in
        # first-needed order: wq, x chunks, bias+wk, wv, y chunks.
        wq_sb = wpool.tile([128, KD, H], f16, tag="wq")
        nc.sync.dma_start(wq_sb[:], wqT[:].rearrange("(k p) h -> p k h", p=128))

        def emit_acts(dram, b, tagp, nops=None):
            # One [128, KD, S] tile per activation tensor, loaded in
            # k-chunked ops so matmul groups start at partial arrival.
            if nops is None:
                nops = _env("KERNEL_ACT_DMAS", 2)
            t = xpool.tile([128, KD, SQ], f16, tag=tagp, name=f"{tagp}_{b}")
            src = dram[b].rearrange("(k p) s -> p k s", p=128)
            bounds = [KD * i // nops for i in range(nops + 1)]
            for i in range(nops):
                nc.sync.dma_start(
                    t[:, bounds[i] : bounds[i + 1], :],
                    src[:, bounds[i] : bounds[i + 1], :],
                )
            return [t[:, k, :] for k in range(KD)]

        # First-needed DMA order for the cold start: wq | x chunks | wk |
        # y chunks | wv | bias. wv is only read by the V projection
        # (which runs after the K wave has consumed all y chunks) and
        # bias only by DVE bias-adds / the epilogue, so both can trail y
        # without stalling the PE.
        b0_chunks = _env("KERNEL_B0_CHUNKS", 6)
        bias_sb = cpool.tile([128, 2 * JH + H], f32, tag="bias")
        bq_sb = bias_sb[:, 0:JH]
        bk_sb = bias_sb[:, JH : 2 * JH]
        bv_sb = bias_sb[:, 2 * JH : 2 * JH + H]
        wk_sb = wpool.tile([128, KD, H], f16, tag="wk")
        wv_sb = wpool.tile([128, KD, H], f16, tag="wv")
        if reps == 1:
            xts0 = emit_acts(xT, 0, "xt", nops=b0_chunks)
            nc.sync.dma_start(wk_sb[:], wkT[:].rearrange("(k p) h -> p k h", p=128))
            yts0 = emit_acts(yT, 0, "yt", nops=b0_chunks)
            nc.sync.dma_start(bias_sb[:], biasd[:])
            wvr = wvT[:].rearrange("(k p) h -> p k h", p=128)
            half = KD // 2
            nc.sync.dma_start(wv_sb[:, 0:half, :], wvr[:, 0:half, :])
            nc.sync.dma_start(wv_sb[:, half:KD, :], wvr[:, half:KD, :])
        else:
            xts0 = None
            yts0 = None
            nc.sync.dma_start(bias_sb[:], biasd[:])
            nc.sync.dma_start(wk_sb[:], wkT[:].rearrange("(k p) h -> p k h", p=128))
            nc.sync.dma_start(wv_sb[:], wvT[:].rearrange("(k p) h -> p k h", p=128))

        def fill_mms(n, pool, tagp):
            # Dummy matmuls over the warmup scratch: keep PE busy (and
            # its clock ramped) across a short expected DMA-arrival gap.
            # A fresh psum tile per group keeps ring reuse clean.
            if nwarm and n:
                fill_ps = pool.tile([128, 512], f32, tag=tagp, name="fill_ps")
                for _ in range(n):
                    nc.tensor.matmul(
                        fill_ps[:, 0:wcols], warm_sb[:, 0:128], warm_sb[:],
                        start=True, stop=True,
                    )

        def proj_qk_waves(xts, yts, qt_sb, kt_sb, b):
            # Batch-0 cold start: k-outer waves of 4 accumulators chase
            # the arriving input chunks. The K wave draws its psum from
            # the (idle-during-proj) psO ring so it has no dependency on
            # the Q accumulators' drain (which waits on the bias DMA).
            for w_sb, acts, bsb, dst, nm, pool in (
                (wq_sb, xts, bq_sb, qt_sb, "q0", psAS),
                (wk_sb, yts, bk_sb, kt_sb, "k0", psO),
            ):
                tagp = "st" if pool is psAS else "o"
                accs = [
                    pool.tile([128, 512], f32, tag=tagp, name=f"pw{nm}{i}")
                    for i in range(4)
                ]
                for k in range(KD):
                    for i, (j, hv) in enumerate(
                        (j, hv) for j in range(JH) for hv in range(2)
                    ):
                        nc.tensor.matmul(
                            accs[i][:],
                            w_sb[:, k, 128 * j : 128 * (j + 1)],
                            acts[k][:, 512 * hv : 512 * (hv + 1)],
                            start=(k == 0),
                            stop=(k == KD - 1),
                        )
                for i, (j, hv) in enumerate(
                    (j, hv) for j in range(JH) for hv in range(2)
                ):
                    nc.vector.tensor_scalar_add(
                        dst[:, j, 512 * hv : 512 * (hv + 1)],
                        accs[i][:],
                        bsb[:, j : j + 1],
                    )
                if nm == "q0":
                    fill_mms(_env("KERNEL_KFILL", 2), psO, "o")
            fill_mms(_env("KERNEL_VFILL", 1), psO, "o")

        def proj_qk_inner(xts, yts, qt_sb, kt_sb, b):
            # Steady state: inputs prefetched, k-inner keeps one psum
            # accumulator live at a time.
            kd_eff = _env("KERNEL_PROJ_KD", KD)  # debug: shorten k-chain
            for w_sb, acts, bsb, dst in (
                (wq_sb, xts, bq_sb, qt_sb),
                (wk_sb, yts, bk_sb, kt_sb),
            ):
                for j in range(JH):
                    for hv in range(2):
                        pp = psProj.tile([128, 512], f32, tag=proj_tag, name=f"pp{b}")
                        for k in range(kd_eff):
                            nc.tensor.matmul(
                                pp[:],
                                w_sb[:, k, 128 * j : 128 * (j + 1)],
                                acts[k][:, 512 * hv : 512 * (hv + 1)],
                                start=(k == 0),
                                stop=(k == kd_eff - 1),
                            )
                        nc.vector.tensor_scalar_add(
                            dst[:, j, 512 * hv : 512 * (hv + 1)],
                            pp[:],
                            bsb[:, j : j + 1],
                        )

        def emit_batch(b, xts, yts, cold=False):
            qt_sb = qkv.tile([128, JH, SQ], f16, tag="qt", name=f"qt_{b}")
            kt_sb = qkv.tile([128, JH, SKV], f16, tag="kt", name=f"kt_{b}")
            v_sb = qkv.tile([128, TS, H + 1], f16, tag="v", name=f"v_{b}")

            if cold:
                proj_qk_waves(xts, yts, qt_sb, kt_sb, b)
            else:
                proj_qk_inner(xts, yts, qt_sb, kt_sb, b)

            # V projection (no bias; folded into the epilogue): V[t, h]
            kdv_eff = _env("KERNEL_VPROJ_KD", KD)  # debug: shorten k-chain
            if _env("KERNEL_ONES_MERGED", 0):
                # Ones column (softmax denominator) for all TS tiles in one
                # strided memset, off the critical path.
                nc.vector.memset(v_sb[:, :, H : H + 1], 1.0)
            for t in range(TS):
                pv = psProj.tile([128, H], f32, tag=proj_tag, name=f"pv{b}")
                for k in range(kdv_eff):
                    nc.tensor.matmul(
                        pv[:],
                        yts[k][:, 128 * t : 128 * (t + 1)],
                        wv_sb[:, k, :],
                        start=(k == 0),
                        stop=(k == kdv_eff - 1),
                    )
                nc.scalar.activation(v_sb[:, t, 0:H], pv[:], Copy)
                if not _env("KERNEL_ONES_MERGED", 0):
                    nc.vector.memset(v_sb[:, t, H : H + 1], 1.0)

            # Attention: flat software pipeline over (sb, t). The P@V
            # matmuls trail the S^T matmuls by one step — across s-block
            # boundaries too — so PE always has exp-independent work in
            # flight while ACT computes exp(t).
            pts = {}
            ops_by_sb = {}

            def emit_st(sb, t):
                st = psAS.tile([128, 512], f32, tag="st", name=f"st{b}")
                for j2 in range(JH):
                    nc.tensor.matmul(
                        st[:],
                        kt_sb[:, j2, 128 * t : 128 * (t + 1)],
                        qt_sb[:, j2, 512 * sb : 512 * (sb + 1)],
                        start=(j2 == 0),
                        stop=(j2 == JH - 1),
                    )
                pt = ptp.tile([128, 512], f16, tag="pt", name=f"pt{b}")
                nc.scalar.activation(pt[:], st[:], Exp, scale=float(H) ** -0.5)
                pts[(sb, t)] = pt

            def emit_epilogue_j(sb, j, acc, last):
                # out_j = pv_j / denom_j + bv, shipped per-j for the
                # final block (short tail drain), staged into one big
                # tile + single DMA otherwise.
                rec = smallp.tile([128, 1], f32, tag="rec", name=f"rec{b}")
                nc.vector.reciprocal(rec[:], acc[:, H : H + 1])
                if last:
                    ot = outp.tile([128, H], f32, tag="otl", name=f"otl{b}_{j}")
                    nc.vector.scalar_tensor_tensor(
                        ot[:], acc[:, 0:H], rec[:], bv_sb[:], op0=mult, op1=add
                    )
                    if os.environ.get("KERNEL_TAIL_ENG", "mixed") == "mixed":
                        eng = (nc.sync, nc.scalar, nc.sync, nc.scalar)[j]
                    else:
                        eng = nc.sync
                    si = 4 * sb + j
                    eng.dma_start(outd[b, 128 * si : 128 * (si + 1), :], ot[:])
                    return None
                return rec

            def emit_epilogue(sb, ops):
                ot = outp.tile([128, 4, H], f32, tag="ot", name=f"ot{b}")
                for j in range(4):
                    rec = emit_epilogue_j(sb, j, ops[j], last=False)
                    nc.vector.scalar_tensor_tensor(
                        ot[:, j, :], ops[j][:, 0:H], rec[:], bv_sb[:],
                        op0=mult, op1=add,
                    )
                oeng = nc.scalar if os.environ.get("KERNEL_OUT_ENG") == "scalar" else nc.sync
                oeng.dma_start(
                    outd[b, 512 * sb : 512 * (sb + 1), :].rearrange(
                        "(j p) h -> p j h", p=128
                    ),
                    ot[:],
                )

            def emit_o(sb, t):
                if t == 0:
                    ops_by_sb[sb] = [
                        psO.tile([128, H + 1], f32, tag="o", name=f"op{b}_{sb}_{j}")
                        for j in range(4)
                    ]
                ops = ops_by_sb[sb]
                for j in range(4):
                    nc.tensor.matmul(
                        ops[j][:],
                        pts[(sb, t)][:, 128 * j : 128 * (j + 1)],
                        v_sb[:, t, :],
                        start=(t == 0),
                        stop=(t == TS - 1),
                    )
                del pts[(sb, t)]
                if t == TS - 1:
                    emit_epilogue(sb, ops)
                    del ops_by_sb[sb]

            def emit_o_jmajor(sb):
                # Tail of the kernel: finish each 128-row output strip
                # (full t-chain, epilogue, DMA) before starting the next,
                # so the drain after the last matmul is one strip deep.
                for j in range(4):
                    acc = psO.tile([128, H + 1], f32, tag="o", name=f"oj{b}_{j}")
                    for t in range(TS):
                        nc.tensor.matmul(
                            acc[:],
                            pts[(sb, t)][:, 128 * j : 128 * (j + 1)],
                            v_sb[:, t, :],
                            start=(t == 0),
                            stop=(t == TS - 1),
                        )
                    emit_epilogue_j(sb, j, acc, last=True)
                for t in range(TS):
                    del pts[(sb, t)]

            if _env("KERNEL_SKIP_ATTN", 0):
                return
            depth = _env("KERNEL_ATTN_DEPTH", 2)
            jmajor_last = b == BL - 1 and reps == 1 and _env("KERNEL_JMAJOR_TAIL", 1)
            seq = [(sb, t) for sb in range(SB) for t in range(TS)]
            if jmajor_last:
                # Flat pipeline over every S^T; trailing emit_o only for
                # the non-final s-blocks (the final one drains j-major,
                # and its S^T stream keeps PE busy over the pipeline
                # boundary).
                nhead = (SB - 1) * TS
                assert nhead + depth <= len(seq)
                for i, (sb, t) in enumerate(seq):
                    emit_st(sb, t)
                    if depth <= i < nhead + depth:
                        emit_o(*seq[i - depth])
                emit_o_jmajor(SB - 1)
            else:
                for i, (sb, t) in enumerate(seq):
                    emit_st(sb, t)
                    if i >= depth:
                        emit_o(*seq[i - depth])
                for i in range(len(seq) - depth, len(seq)):
                    emit_o(*seq[i])

        def emit_body(first=False):
            for b in range(BL):
                if first and b == 0:
                    emit_batch(0, xts0, yts0, cold=bool(_env("KERNEL_COLD_WAVES", 1)))
                else:
                    emit_batch(b, emit_acts(xT, b, "xt"), emit_acts(yT, b, "yt"))

        if reps == 1:
            emit_body(first=True)
            for _ in range(_env("KERNEL_EXTRA_BODIES", 0)):
                emit_body()
        else:
            # Device-side repetition for wall-clock benchmarking (the
            # per-call dispatch overhead through axon is ~80ms, far above
            # the kernel's span; the R-vs-1 slope isolates HW time).
            nb = _env("KERNEL_BODIES_PER_ITER", 1)
            with tc.For_i(0, reps, 1):
                for _ in range(nb):
                    emit_body()

    nc.compile()
    return nc


def _get_nc(reps=1):
    key = ("nc", reps)
    if key not in _cached:
        _cached[key] = _build_nc(reps)
    return _cached[key]


def make_in_maps(x, y, Wq, bq, Wk, bk, Wv, bv):

    f16 = np.float16
    wq_h = np.ascontiguousarray(Wq.T).astype(f16)  # [D, H]
    wk_h = np.ascontiguousarray(Wk.T).astype(f16)
    wv_h = np.ascontiguousarray(Wv.T).astype(f16)
    bias_h = np.empty((128, 2 * JH + H), np.float32)
    bias_h[:, 0:JH] = np.asarray(bq, np.float32).reshape(JH, 128).T
    bias_h[:, JH : 2 * JH] = np.asarray(bk, np.float32).reshape(JH, 128).T
    bias_h[:, 2 * JH :] = np.asarray(bv, np.float32)[None, :]

    in_maps = []
    for c in range(N_CORES):
        xs = np.asarray(x[BL * c : BL * (c + 1)])  # [BL, Sq, D]
        ys = np.asarray(y[BL * c : BL * (c + 1)])
        in_maps.append(
            {
                "xT": np.ascontiguousarray(xs.transpose(0, 2, 1)).astype(f16),
                "yT": np.ascontiguousarray(ys.transpose(0, 2, 1)).astype(f16),
                "wqT": wq_h,
                "wkT": wk_h,
                "wvT": wv_h,
                "biases": bias_h,
            }
        )
    return in_maps


def kernel(x, y, Wq, bq, Wk, bk, Wv, bv):
    from concourse.bass_utils import run_bass_kernel_spmd

    nc = _get_nc()
    in_maps = make_in_maps(x, y, Wq, bq, Wk, bk, Wv, bv)
    bkr = run_bass_kernel_spmd(
        nc,
        in_maps,
        list(range(N_CORES)),
        trace=bool(os.environ.get("KERNEL_TRACE")),
    )
    _cached["last_results"] = bkr
    return np.concatenate([r["out"] for r in bkr.results], axis=0)

